# revision 21
# baseline (speedup 1.0000x reference)
"""Trainium2 Bass kernel for nn_AttnMoveModel (dense_transformer).

Strategy (8 NeuronCores):
  - Only the `curr` path of the reference affects the output (hist self-attn and
    cross-attn results are dead), so only that path is computed.
  - Attention is data-parallel over batch (4 of 32 batches per core).
  - The vocab projection (gathered @ emb[2:].T) is tensor-parallel, column-split
    over the vocab (5120 padded columns per core), with an AllGather of the
    gathered activations (G) before it.
  - All matmul operands are fp16/bf16 (PSUM accumulation stays fp32): small-N
    attention matmuls run at 1 cyc/row instead of fp32r's 4, transposes at 1
    instead of 2, and DMA bytes are halved.
  - Attention scores are computed TRANSPOSED (K^T as weights, Q^T streaming) so
    the exp output is P^T directly -- no per-head PE transposes.  Softmax row
    sums come from a ones-column appended to V; normalization is a DVE
    broadcast-multiply folded in front of one tanh per batch.
  - Phase A is software-pipelined over batches: batch b's transposes /
    projections / scores are emitted before batch b-1's AV+tanh+select, so the
    in-order PE never stalls on the scalar-engine softmax of the same batch.
  - The device stores e = exp(scores - 30) in bf16 (streamed to DRAM during the
    matmul phase) plus per-core per-chunk exp-sums.  The host computes
    log_softmax = ln(e) - ln(sum_cores sum_chunks sums) in numpy: no raw-score
    buffering, no device-side subtract pass, and no second AllGather.

Host-side prep (inside kernel()): shard indices/batches, pre-transpose weights
and the emb vocab shard to fp16, build one-hot selection matrices from
mask_pos, positional-encoding table; post: ln(e) - ln(S).
"""
import math
import sys

sys.path.insert(0, "/opt/trn_rl_repo")

import numpy as np

import concourse.bass as bass
import concourse.mybir as mybir
import concourse.tile as tile
from concourse.tile import add_dep_helper
from concourse import bacc
from concourse.bass_utils import run_bass_kernel_spmd

FP32 = mybir.dt.float32
FP16 = mybir.dt.float16
BF16 = mybir.dt.bfloat16
INT32 = mybir.dt.int32
ACTF = mybir.ActivationFunctionType

N_CORES = 8
B, S, D, H, DH = 32, 128, 512, 8, 64
B_LOC = B // N_CORES              # 4 batches per core
NM = 16                           # mask positions per batch
I_LOC = B_LOC * NM                # 64 gathered rows per core
I_TOT = B * NM                    # 512 gathered rows total
GRID = 40000
VOCAB = GRID - 2                  # 39998 candidate rows
VSH = 5120                        # padded vocab shard per core (8*5120 >= VOCAB)
VCH = 512                         # vocab chunk (matmul N)
NCH = VSH // VCH                  # 10 chunks
KD = D // 128                     # 4 contraction tiles
DHP = DH + 1                      # V' head width (ones column for row sums)
SH_ATT = 15.0                     # exp shift for attention softmax
SH_SC = 30.0                      # exp shift for final scores


def _positional_embedding(d_model, max_len):
    pe = np.zeros((max_len, d_model), dtype=np.float32)
    position = np.arange(max_len, dtype=np.float32)[:, None]
    div_term = np.exp(np.arange(0, d_model, 2, dtype=np.float32) * -(math.log(10000.0) / d_model))
    pe[:, 0::2] = np.sin(position * div_term)
    pe[:, 1::2] = np.cos(position * div_term)
    return pe


def build(sim_local=False):
    nc = bacc.Bacc("TRN2", target_bir_lowering=False, debug=False, num_devices=N_CORES)

    # ---- I/O ----
    emb16 = nc.dram_tensor("emb16", [GRID, D], FP16, kind="ExternalInput")
    candT = nc.dram_tensor("candT", [D, VSH], FP16, kind="ExternalInput")
    idx = nc.dram_tensor("idx", [B_LOC * S], INT32, kind="ExternalInput")
    peT = nc.dram_tensor("peT", [D, S], FP32, kind="ExternalInput")
    wqt = nc.dram_tensor("wqt", [D, D], FP16, kind="ExternalInput")
    wkt = nc.dram_tensor("wkt", [D, D], FP16, kind="ExternalInput")
    wvt = nc.dram_tensor("wvt", [D, D], FP16, kind="ExternalInput")
    t2wt = nc.dram_tensor("t2wt", [D, D], FP16, kind="ExternalInput")
    bias3 = nc.dram_tensor("bias3", [D, 3], FP32, kind="ExternalInput")  # bq|bk|t2b
    bv = nc.dram_tensor("bv", [D], FP32, kind="ExternalInput")
    sel = nc.dram_tensor("sel", [B_LOC, S, NM], FP16, kind="ExternalInput")
    ident = nc.dram_tensor("ident", [128, 128], FP16, kind="ExternalInput")
    out_e = nc.dram_tensor("out_e", [I_TOT, VSH], BF16, kind="ExternalOutput")
    out_s = nc.dram_tensor("out_s", [I_TOT, NCH], FP32, kind="ExternalOutput")

    with tile.TileContext(nc) as tc:
        with (
            tc.tile_pool(name="const", bufs=1) as constp,
            tc.tile_pool(name="small", bufs=2) as smallp,
            tc.tile_pool(name="dram", bufs=1, space="DRAM") as dramp,
        ):
            # ================= constants / persistent =================
            # idx first: the gathers are the head critical path
            idx_sb = constp.tile([S, B_LOC], INT32)
            nc.sync.dma_start(out=idx_sb[:, :],
                              in_=idx.ap().rearrange("(b s) -> s b", s=S))
            ident_sb = constp.tile([128, 128], FP16)
            nc.sync.dma_start(out=ident_sb[:, :], in_=ident.ap())
            peT_sb = constp.tile([128, KD, S], FP32)  # [d%128, kd, s]
            nc.sync.dma_start(out=peT_sb[:, :, :],
                              in_=peT.ap().rearrange("(kd p) s -> p kd s", p=128))
            b3_sb = constp.tile([128, KD, 3], FP32)   # [:, kj, 0]=bq [*,1]=bk [*,2]=t2b
            nc.sync.dma_start(out=b3_sb[:, :, :],
                              in_=bias3.ap().rearrange("(kj p) t -> p kj t", p=128))
            bvb_sb = constp.tile([128, D], FP32)
            nc.sync.dma_start(out=bvb_sb[:, :],
                              in_=bv.ap().rearrange("(one j) -> one j", one=1).to_broadcast([128, D]))
            sel_sb = constp.tile([S, B_LOC, NM], FP16)
            nc.sync.dma_start(out=sel_sb[:, :, :],
                              in_=sel.ap().rearrange("b s m -> s b m"))
            shatt_sb = constp.tile([128, 1], FP32)
            nc.vector.memset(shatt_sb[:, :], -SH_ATT)
            shsc_sb = constp.tile([128, 1], FP32)
            nc.vector.memset(shsc_sb[:, :], -SH_SC)

            # persistent across phases
            GT_sb = constp.tile([128, KD, N_CORES, I_LOC], FP16)  # [d%128, kd, c, i]
            sums_sb = constp.tile([128, KD, NCH], FP32)           # per-chunk exp sums
            cand_sb = constp.tile([128, KD, VSH], FP16)           # full vocab shard

            ag_g_in = dramp.tile([D * I_LOC], FP16)
            ag_g_out = dramp.tile([N_CORES * D * I_LOC], FP16, addr_space="Shared")

            # ================= Phase A: gather + self-attention =================
            with (
                tc.tile_pool(name="wts", bufs=1) as wtsp,
                tc.tile_pool(name="acts", bufs=1) as actsp,
                tc.tile_pool(name="gath", bufs=4) as gathp,
                tc.tile_pool(name="ph", bufs=2) as php,
                tc.tile_pool(name="ps_qkv", bufs=2, space="PSUM") as ps_qkv,
                tc.tile_pool(name="ps_t", bufs=2, space="PSUM") as ps_t,
                tc.tile_pool(name="ps_sav", bufs=4, space="PSUM") as ps_sav,
            ):
                # gathers go on the gpsimd (SWDGE) queue -- independent of the
                # HWDGE weight loads below.
                gath_tiles = []
                gath_insts = []
                with tc.high_priority():
                    for b in range(B_LOC):
                        g_t = gathp.tile([S, D], FP16, tag="gather")
                        g_i = nc.gpsimd.indirect_dma_start(
                            out=g_t[:, :], out_offset=None,
                            in_=emb16.ap(),
                            in_offset=bass.IndirectOffsetOnAxis(ap=idx_sb[:, b:b + 1], axis=0),
                        )
                        gath_tiles.append(g_t)
                        gath_insts.append(g_i)

                wqt_sb = wtsp.tile([128, KD, D], FP16, tag="w", bufs=4)  # [d%128, kd, j]
                nc.sync.dma_start(out=wqt_sb[:, :, :],
                                  in_=wqt.ap().rearrange("(kd p) j -> p kd j", p=128))
                wkt_sb = wtsp.tile([128, KD, D], FP16, tag="w", bufs=4)
                nc.sync.dma_start(out=wkt_sb[:, :, :],
                                  in_=wkt.ap().rearrange("(kd p) j -> p kd j", p=128))
                wvt_sb = wtsp.tile([128, KD, D], FP16, tag="w", bufs=4)
                nc.sync.dma_start(out=wvt_sb[:, :, :],
                                  in_=wvt.ap().rearrange("(kd p) j -> p kd j", p=128))
                t2wt_sb = wtsp.tile([128, KD, D], FP16, tag="w", bufs=4)
                nc.sync.dma_start(out=t2wt_sb[:, :, :],
                                  in_=t2wt.ap().rearrange("(kd p) j -> p kd j", p=128))
                # vocab shard prefetch: artificially held behind the gathers so
                # the serial DMA queue serves the latency-critical tiles first
                # (SP issues in order, so one dep on the first chunk suffices).
                for v in range(NCH):
                    c_i = nc.sync.dma_start(
                        out=cand_sb[:, :, v * VCH:(v + 1) * VCH],
                        in_=candT.ap()[:, v * VCH:(v + 1) * VCH]
                            .rearrange("(kd p) n -> p kd n", p=128),
                    )
                    if v == 0:
                        add_dep_helper(c_i.ins, gath_insts[-1].ins,
                                       reason="cand prefetch yields DMA queue to gathers")

                currT_sb = actsp.tile([128, KD, B_LOC * S], FP16)  # [d%128, kd, (b,s)]
                QT_sb = actsp.tile([128, KD, B_LOC * S], FP16)  # [j%128, kj, (b,s)]
                KT_sb = actsp.tile([128, KD, B_LOC * S], FP16)
                V_sb = actsp.tile([128, B_LOC, H, DHP], BF16)  # [s, b, h, dh+1]
                nc.vector.memset(V_sb[:, :, :, DH:DHP], 1.0)
                th_sb = actsp.tile([128, B_LOC, D], FP16)      # tanh(attn) [s, b, j]
                thsel_sb = actsp.tile([128, KD, I_LOC], FP16)  # [d%128, kd, i]

                # --- stage 1 of the batch pipeline: transpose+pe, QKV, S^T ---
                def stage1(b):
                    bs = slice(b * S, (b + 1) * S)
                    for kd in range(KD):
                        tp_ps = ps_t.tile([128, 128], FP16, tag="t")
                        nc.tensor.transpose(tp_ps[:, :],
                                            gath_tiles[b][:, kd * 128:(kd + 1) * 128],
                                            ident_sb[:, :])
                        nc.vector.tensor_add(
                            out=currT_sb[:, kd, b * S:(b + 1) * S],
                            in0=tp_ps[:, :], in1=peT_sb[:, kd, :])
                    q_ps = ps_qkv.tile([128, KD * S], FP32, tag="qkv")
                    k_ps = ps_qkv.tile([128, KD * S], FP32, tag="qkv")
                    for kj in range(KD):
                        for kd in range(KD):
                            nc.tensor.matmul(q_ps[:, kj * S:(kj + 1) * S],
                                             wqt_sb[:, kd, kj * 128:(kj + 1) * 128],
                                             currT_sb[:, kd, bs],
                                             start=(kd == 0), stop=(kd == KD - 1))
                        for kd in range(KD):
                            nc.tensor.matmul(k_ps[:, kj * S:(kj + 1) * S],
                                             wkt_sb[:, kd, kj * 128:(kj + 1) * 128],
                                             currT_sb[:, kd, bs],
                                             start=(kd == 0), stop=(kd == KD - 1))
                    for kj in range(KD):
                        nc.vector.tensor_scalar_add(QT_sb[:, kj, bs],
                                                    q_ps[:, kj * S:(kj + 1) * S],
                                                    b3_sb[:, kj, 0:1])
                        nc.vector.tensor_scalar_add(KT_sb[:, kj, bs],
                                                    k_ps[:, kj * S:(kj + 1) * S],
                                                    b3_sb[:, kj, 1:2])
                    v_ps = ps_qkv.tile([128, D], FP32, tag="qkv")
                    for kd in range(KD):
                        nc.tensor.matmul(v_ps[:, :],
                                         currT_sb[:, kd, bs],
                                         wvt_sb[:, kd, :],
                                         start=(kd == 0), stop=(kd == KD - 1))
                    nc.vector.tensor_add(
                        out=V_sb[:, b, :, 0:DH],
                        in0=v_ps[:, :].rearrange("p (h d) -> p h d", d=DH),
                        in1=bvb_sb[:, :].rearrange("p (h d) -> p h d", d=DH))
                    # S^T scores: heads with row-half 0 in one PSUM bank, 1 in
                    # the other (same-bank matmuls must share a row group)
                    s_ps0 = ps_sav.tile([128, 4 * S], FP32, tag="sav")
                    s_ps1 = ps_sav.tile([128, 4 * S], FP32, tag="sav")
                    for h in range(H):
                        kj, half = divmod(h, 2)
                        rows = slice(half * 64, (half + 1) * 64)
                        s_ps_h = s_ps0 if half == 0 else s_ps1
                        o = (h // 2) * S
                        nc.tensor.matmul(s_ps_h[:, o:o + S],
                                         KT_sb[rows, kj, bs], QT_sb[rows, kj, bs],
                                         start=True, stop=True)
                    p_sb = php.tile([128, H * S], BF16, tag="p", bufs=3)  # P^T slots
                    nc.scalar.activation(p_sb[:, 0:4 * S], s_ps0[:, :], ACTF.Exp,
                                         bias=shatt_sb[:, :1])
                    nc.scalar.activation(p_sb[:, 4 * S:8 * S], s_ps1[:, :], ACTF.Exp,
                                         bias=shatt_sb[:, :1])
                    return p_sb

                # --- stage 2: AV', normalize, tanh, select ---
                def stage2(b, p_sb):
                    av_ps0 = ps_sav.tile([128, 4 * DHP], FP32, tag="sav")
                    av_ps1 = ps_sav.tile([128, 4 * DHP], FP32, tag="sav")
                    last_av = [None, None]
                    for h in range(H):
                        sl = (h % 2) * 4 + h // 2  # slot index of head h in p_sb
                        av_ps = av_ps0 if h < 4 else av_ps1
                        last_av[h // 4] = nc.tensor.matmul(
                            av_ps[:, (h % 4) * DHP:(h % 4 + 1) * DHP],
                            p_sb[:, sl * S:(sl + 1) * S],
                            V_sb[:, b, h, :],
                            start=True, stop=True)
                    rec_sb = smallp.tile([128, H], FP32, tag="rec")
                    for g in range(2):
                        av_ps = av_ps0 if g == 0 else av_ps1
                        r_i = nc.vector.reciprocal(
                            rec_sb[:, g * 4:(g + 1) * 4],
                            av_ps[:, :].rearrange("p (h d) -> p h d", d=DHP)[:, :, DH])
                        add_dep_helper(r_i.ins, last_av[g].ins, reason="rec after AV bank")
                    avn_sb = php.tile([128, H, DH], FP32, tag="avn", bufs=3)
                    for g in range(2):
                        av_ps = av_ps0 if g == 0 else av_ps1
                        m_i = nc.vector.tensor_mul(
                            out=avn_sb[:, g * 4:(g + 1) * 4, :],
                            in0=av_ps[:, :].rearrange("p (h d) -> p h d", d=DHP)[:, :, 0:DH],
                            in1=rec_sb[:, g * 4:(g + 1) * 4]
                                .rearrange("p (h one) -> p h one", one=1)
                                .to_broadcast([128, 4, DH]))
                        add_dep_helper(m_i.ins, last_av[g].ins,
                                       reason="attn bank read after all AV writes")
                    nc.scalar.activation(th_sb[:, b, :],
                                         avn_sb[:, :, :].rearrange("p h d -> p (h d)"),
                                         ACTF.Tanh)
                    # select this batch's mask positions: [d%128, kd, NM]
                    ts_ps = ps_t.tile([128, KD, NM], FP32, tag="t")
                    for kd in range(KD):
                        nc.tensor.matmul(ts_ps[:, kd, :],
                                         th_sb[:, b, kd * 128:(kd + 1) * 128],
                                         sel_sb[:, b, :],
                                         start=True, stop=True)
                    nc.vector.tensor_copy(
                        out=thsel_sb[:, :, :]
                            .rearrange("p kd (b m) -> p kd b m", b=B_LOC)[:, :, b, :],
                        in_=ts_ps[:, :, :])

                prev = None
                for b in range(B_LOC):
                    p_sb = stage1(b)
                    if prev is not None:
                        stage2(*prev)
                    prev = (b, p_sb)
                stage2(*prev)

                # t2 projection -> G^T [d, i_loc], all four mj blocks in one
                # SBUF tile so the AllGather input is a single DMA
                gt_sb = smallp.tile([128, KD, I_LOC], FP16, tag="gt", bufs=1)
                for mj in range(KD):
                    g_ps = ps_t.tile([128, I_LOC], FP32, tag="t")
                    for kd in range(KD):
                        nc.tensor.matmul(g_ps[:, :],
                                         t2wt_sb[:, kd, mj * 128:(mj + 1) * 128],
                                         thsel_sb[:, kd, :],
                                         start=(kd == 0), stop=(kd == KD - 1))
                    nc.vector.tensor_scalar_add(gt_sb[:, mj, :], g_ps[:, :],
                                                b3_sb[:, mj, 2:3])
                nc.sync.dma_start(
                    out=ag_g_in[:].rearrange("(mj p i) -> p mj i", p=128, mj=KD),
                    in_=gt_sb[:, :, :])

                # ---- AllGather G (the only collective) ----
                if sim_local:
                    nc.sync.dma_start(
                        out=ag_g_out[:].rearrange("(c x) -> c x", c=N_CORES),
                        in_=ag_g_in[:].rearrange("(one x) -> one x", one=1)
                            .to_broadcast([N_CORES, D * I_LOC]))
                else:
                    nc.gpsimd.collective_compute(
                        "AllGather", mybir.AluOpType.bypass,
                        replica_groups=[list(range(N_CORES))],
                        ins=[ag_g_in[:].opt()], outs=[ag_g_out[:].opt()],
                    )
                ag_g_view = ag_g_out[:].rearrange("(c kd p i) -> p kd c i", p=128, kd=KD, i=I_LOC)
                for kd in range(KD):
                    nc.sync.dma_start(out=GT_sb[:, kd, :, :], in_=ag_g_view[:, kd, :, :])

            # ================= Phase B: scores -> exp -> DMA out =================
            with (
                tc.tile_pool(name="ps_sc", bufs=8, space="PSUM") as ps_sc,
                tc.tile_pool(name="esb", bufs=4) as esbp,
            ):
                gt_view = GT_sb[:, :, :, :].rearrange("p kd c i -> p kd (c i)")
                for v in range(NCH):
                    for mi in range(KD):
                        sc_ps = ps_sc.tile([128, VCH], FP32, tag="sc")
                        for kd in range(KD):
                            nc.tensor.matmul(sc_ps[:, :],
                                             gt_view[:, kd, mi * 128:(mi + 1) * 128],
                                             cand_sb[:, kd, v * VCH:(v + 1) * VCH],
                                             start=(kd == 0), stop=(kd == KD - 1))
                        e_sb = esbp.tile([128, VCH], BF16, tag="e")
                        nc.scalar.activation(e_sb[:, :], sc_ps[:, :],
                                             ACTF.Exp, bias=shsc_sb[:, :1])
                        nc.vector.reduce_sum(sums_sb[:, mi, v:v + 1], e_sb[:, :],
                                             axis=mybir.AxisListType.X)
                        nc.sync.dma_start(
                            out=out_e.ap()[mi * 128:(mi + 1) * 128, v * VCH:(v + 1) * VCH],
                            in_=e_sb[:, :])

                # raw per-chunk sums -> host does the final reduction
                nc.sync.dma_start(
                    out=out_s.ap().rearrange("(mi p) v -> p mi v", p=128),
                    in_=sums_sb[:, :, :])
    nc.compile()
    return nc


_NC_CACHE = None


def _get_nc():
    global _NC_CACHE
    if _NC_CACHE is None:
        _NC_CACHE = build()
    return _NC_CACHE


def prepare_in_maps(inputs):
    emb = np.asarray(inputs["emb"], dtype=np.float32)
    emb16 = np.ascontiguousarray(emb.astype(np.float16))
    mask_curr = np.asarray(inputs["mask_curr_traj_grid"]).astype(np.int32)
    mask_pos = np.asarray(inputs["mask_pos"]).astype(np.int32)
    wqt = np.ascontiguousarray(np.asarray(inputs["c_wq"], dtype=np.float32).T.astype(np.float16))
    wkt = np.ascontiguousarray(np.asarray(inputs["c_wk"], dtype=np.float32).T.astype(np.float16))
    wvt = np.ascontiguousarray(np.asarray(inputs["c_wv"], dtype=np.float32).T.astype(np.float16))
    t2wt = np.ascontiguousarray(np.asarray(inputs["t2_w"], dtype=np.float32).T.astype(np.float16))
    bias3 = np.ascontiguousarray(np.stack(
        [np.asarray(inputs["c_bq"], dtype=np.float32),
         np.asarray(inputs["c_bk"], dtype=np.float32),
         np.asarray(inputs["t2_b"], dtype=np.float32)], axis=1))
    bv = np.asarray(inputs["c_bv"], dtype=np.float32)
    peT = np.ascontiguousarray(_positional_embedding(D, S).T)
    ident = np.eye(128, dtype=np.float16)

    cand = emb[2:]
    in_maps = []
    for c in range(N_CORES):
        lo = c * VSH
        hi = min((c + 1) * VSH, VOCAB)
        shard = np.zeros((VSH, D), dtype=np.float16)
        shard[: hi - lo] = cand[lo:hi].astype(np.float16)
        candT_c = np.ascontiguousarray(shard.T)
        mp = mask_pos[c * B_LOC:(c + 1) * B_LOC]  # [B_LOC, NM]
        sel_c = np.zeros((B_LOC, S, NM), dtype=np.float16)
        for b in range(B_LOC):
            sel_c[b, mp[b], np.arange(NM)] = 1.0
        in_maps.append(dict(
            emb16=emb16,
            candT=candT_c,
            idx=np.ascontiguousarray(mask_curr[c * B_LOC:(c + 1) * B_LOC].reshape(-1)),
            peT=peT, wqt=wqt, wkt=wkt, wvt=wvt, t2wt=t2wt,
            bias3=bias3, bv=bv,
            sel=sel_c, ident=ident,
        ))
    return in_maps


def assemble_output(results):
    # host-side log_softmax epilogue: out = ln(e) - ln(sum_c sum_v S_cv - pad)
    n_pad = N_CORES * VSH - VOCAB
    s_tot = np.zeros(I_TOT, dtype=np.float64)
    for c in range(N_CORES):
        s_tot += np.asarray(results[c]["out_s"], dtype=np.float64).sum(axis=1)
    s_tot -= n_pad * math.exp(-SH_SC)   # padded vocab columns contribute exp(-SH_SC)
    ln_s = np.log(s_tot).astype(np.float32)[:, None]
    parts = []
    for c in range(N_CORES):
        lo = c * VSH
        hi = min((c + 1) * VSH, VOCAB)
        e = np.asarray(results[c]["out_e"][:, : hi - lo], dtype=np.float32)
        parts.append(np.log(np.maximum(e, 1e-38)) - ln_s)
    return np.ascontiguousarray(np.concatenate(parts, axis=1))


def kernel(**inputs):
    nc = _get_nc()
    in_maps = prepare_in_maps(inputs)
    res = run_bass_kernel_spmd(nc, in_maps, core_ids=list(range(N_CORES)))
    return assemble_output(res.results)


# revision 28
# speedup vs baseline: 1.0502x; 1.0502x over previous
"""Trainium2 Bass kernel for nn_AttnMoveModel (dense_transformer).

Strategy (8 NeuronCores):
  - Only the `curr` path of the reference affects the output (hist self-attn and
    cross-attn results are dead), so only that path is computed.
  - Attention is data-parallel over batch (4 of 32 batches per core).
  - The vocab projection (gathered @ emb[2:].T) is tensor-parallel, column-split
    over the vocab (5120 padded columns per core), with an AllGather of the
    gathered activations (G) before it.
  - All matmul operands are fp16/bf16 (PSUM accumulation stays fp32): small-N
    attention matmuls run at 1 cyc/row instead of fp32r's 4, transposes at 1
    instead of 2, and DMA bytes are halved.
  - Attention scores are computed TRANSPOSED (K^T as weights, Q^T streaming) so
    the exp output is P^T directly -- no per-head PE transposes.  Softmax row
    sums come from a ones-column appended to V; normalization is a DVE
    broadcast-multiply folded in front of one tanh per batch.
  - Phase A is software-pipelined over batches: batch b's transposes /
    projections / scores are emitted before batch b-1's AV+tanh+select, so the
    in-order PE never stalls on the scalar-engine softmax of the same batch.
  - The device stores e = exp(scores - 30) in bf16 (streamed to DRAM during the
    matmul phase) plus per-core per-chunk exp-sums.  The host computes
    log_softmax = ln(e) - ln(sum_cores sum_chunks sums) in numpy: no raw-score
    buffering, no device-side subtract pass, and no second AllGather.

Host-side prep (inside kernel()): shard indices/batches, pre-transpose weights
and the emb vocab shard to fp16, build one-hot selection matrices from
mask_pos, positional-encoding table; post: ln(e) - ln(S).
"""
import math
import sys

sys.path.insert(0, "/opt/trn_rl_repo")

import numpy as np

import concourse.bass as bass
import concourse.mybir as mybir
import concourse.tile as tile
from concourse.tile import add_dep_helper
from concourse import bacc
from concourse.bass_utils import run_bass_kernel_spmd

FP32 = mybir.dt.float32
FP16 = mybir.dt.float16
BF16 = mybir.dt.bfloat16
INT32 = mybir.dt.int32
ACTF = mybir.ActivationFunctionType

N_CORES = 8
B, S, D, H, DH = 32, 128, 512, 8, 64
B_LOC = B // N_CORES              # 4 batches per core
NM = 16                           # mask positions per batch
I_LOC = B_LOC * NM                # 64 gathered rows per core
I_TOT = B * NM                    # 512 gathered rows total
GRID = 40000
VOCAB = GRID - 2                  # 39998 candidate rows
VSH = 5120                        # padded vocab shard per core (8*5120 >= VOCAB)
VCH = 512                         # vocab chunk (matmul N)
NCH = VSH // VCH                  # 10 chunks
KD = D // 128                     # 4 contraction tiles
DHP = DH + 1                      # V' head width (ones column for row sums)
SH_ATT = 15.0                     # exp shift for attention softmax
SH_SC = 30.0                      # exp shift for final scores


def _positional_embedding(d_model, max_len):
    pe = np.zeros((max_len, d_model), dtype=np.float32)
    position = np.arange(max_len, dtype=np.float32)[:, None]
    div_term = np.exp(np.arange(0, d_model, 2, dtype=np.float32) * -(math.log(10000.0) / d_model))
    pe[:, 0::2] = np.sin(position * div_term)
    pe[:, 1::2] = np.cos(position * div_term)
    return pe


def build(sim_local=False):
    nc = bacc.Bacc("TRN2", target_bir_lowering=False, debug=False, num_devices=N_CORES)

    # ---- I/O ----
    emb16 = nc.dram_tensor("emb16", [GRID, D], FP16, kind="ExternalInput")
    candT = nc.dram_tensor("candT", [D, VSH], FP16, kind="ExternalInput")
    idx = nc.dram_tensor("idx", [B_LOC * S], INT32, kind="ExternalInput")
    peT = nc.dram_tensor("peT", [D, S], FP32, kind="ExternalInput")
    wqt = nc.dram_tensor("wqt", [D, D], FP16, kind="ExternalInput")
    wkt = nc.dram_tensor("wkt", [D, D], FP16, kind="ExternalInput")
    wvt = nc.dram_tensor("wvt", [D, D], FP16, kind="ExternalInput")
    t2wt = nc.dram_tensor("t2wt", [D, D], FP16, kind="ExternalInput")
    bias3 = nc.dram_tensor("bias3", [D, 3], FP32, kind="ExternalInput")  # bq|bk|t2b
    bv = nc.dram_tensor("bv", [D], FP32, kind="ExternalInput")
    sel = nc.dram_tensor("sel", [B_LOC, S, NM], FP16, kind="ExternalInput")
    ident = nc.dram_tensor("ident", [128, 128], FP16, kind="ExternalInput")
    out_e = nc.dram_tensor("out_e", [I_TOT, VSH], BF16, kind="ExternalOutput")

    with tile.TileContext(nc) as tc:
        with (
            tc.tile_pool(name="const", bufs=1) as constp,
            tc.tile_pool(name="small", bufs=2) as smallp,
            tc.tile_pool(name="dram", bufs=1, space="DRAM") as dramp,
        ):
            # ================= constants / persistent =================
            # idx first: the gathers are the head critical path
            idx_sb = constp.tile([S, B_LOC], INT32)
            nc.sync.dma_start(out=idx_sb[:, :],
                              in_=idx.ap().rearrange("(b s) -> s b", s=S))
            ident_sb = constp.tile([128, 128], FP16)
            nc.sync.dma_start(out=ident_sb[:, :], in_=ident.ap())
            peT_sb = constp.tile([128, KD, S], FP32)  # [d%128, kd, s]
            nc.sync.dma_start(out=peT_sb[:, :, :],
                              in_=peT.ap().rearrange("(kd p) s -> p kd s", p=128))
            b3_sb = constp.tile([128, KD, 3], FP32)   # [:, kj, 0]=bq [*,1]=bk [*,2]=t2b
            nc.sync.dma_start(out=b3_sb[:, :, :],
                              in_=bias3.ap().rearrange("(kj p) t -> p kj t", p=128))
            bvb_sb = constp.tile([128, D], FP32)
            nc.sync.dma_start(out=bvb_sb[:, :],
                              in_=bv.ap().rearrange("(one j) -> one j", one=1).to_broadcast([128, D]))
            sel_sb = constp.tile([S, B_LOC, NM], FP16)
            nc.sync.dma_start(out=sel_sb[:, :, :],
                              in_=sel.ap().rearrange("b s m -> s b m"))
            shatt_sb = constp.tile([128, 1], FP32)
            nc.vector.memset(shatt_sb[:, :], -SH_ATT)
            shsc_sb = constp.tile([128, 1], FP32)
            nc.vector.memset(shsc_sb[:, :], -SH_SC)

            # persistent across phases
            GT_sb = constp.tile([128, KD, N_CORES, I_LOC], FP16)  # [d%128, kd, c, i]
            cand_sb = constp.tile([128, KD, VSH], FP16)           # full vocab shard

            ag_g_in = dramp.tile([D * I_LOC], FP16)
            ag_g_out = dramp.tile([N_CORES * D * I_LOC], FP16, addr_space="Shared")

            # ================= Phase A: gather + self-attention =================
            with (
                tc.tile_pool(name="wts", bufs=1) as wtsp,
                tc.tile_pool(name="acts", bufs=1) as actsp,
                tc.tile_pool(name="gath", bufs=4) as gathp,
                tc.tile_pool(name="ph", bufs=2) as php,
                tc.tile_pool(name="ps_qkv", bufs=2, space="PSUM") as ps_qkv,
                tc.tile_pool(name="ps_t", bufs=2, space="PSUM") as ps_t,
                tc.tile_pool(name="ps_sav", bufs=4, space="PSUM") as ps_sav,
            ):
                # gathers go on the gpsimd (SWDGE) queue -- independent of the
                # HWDGE weight loads below.
                gath_tiles = []
                gath_insts = []
                with tc.high_priority():
                    for b in range(B_LOC):
                        g_t = gathp.tile([S, D], FP16, tag="gather")
                        g_i = nc.gpsimd.indirect_dma_start(
                            out=g_t[:, :], out_offset=None,
                            in_=emb16.ap(),
                            in_offset=bass.IndirectOffsetOnAxis(ap=idx_sb[:, b:b + 1], axis=0),
                        )
                        gath_tiles.append(g_t)
                        gath_insts.append(g_i)

                wqt_sb = wtsp.tile([128, KD, D], FP16, tag="w", bufs=4)  # [d%128, kd, j]
                nc.sync.dma_start(out=wqt_sb[:, :, :],
                                  in_=wqt.ap().rearrange("(kd p) j -> p kd j", p=128))
                wkt_sb = wtsp.tile([128, KD, D], FP16, tag="w", bufs=4)
                nc.sync.dma_start(out=wkt_sb[:, :, :],
                                  in_=wkt.ap().rearrange("(kd p) j -> p kd j", p=128))
                wvt_sb = wtsp.tile([128, KD, D], FP16, tag="w", bufs=4)
                nc.sync.dma_start(out=wvt_sb[:, :, :],
                                  in_=wvt.ap().rearrange("(kd p) j -> p kd j", p=128))
                t2wt_sb = wtsp.tile([128, KD, D], FP16, tag="w", bufs=4)
                nc.sync.dma_start(out=t2wt_sb[:, :, :],
                                  in_=t2wt.ap().rearrange("(kd p) j -> p kd j", p=128))
                # vocab shard prefetch: artificially held behind the gathers so
                # the serial DMA queue serves the latency-critical tiles first
                # (SP issues in order, so one dep on the first chunk suffices).
                for v in range(NCH):
                    c_i = nc.sync.dma_start(
                        out=cand_sb[:, :, v * VCH:(v + 1) * VCH],
                        in_=candT.ap()[:, v * VCH:(v + 1) * VCH]
                            .rearrange("(kd p) n -> p kd n", p=128),
                    )
                    if v == 0:
                        add_dep_helper(c_i.ins, gath_insts[-1].ins,
                                       reason="cand prefetch yields DMA queue to gathers")

                currT_sb = actsp.tile([128, KD, B_LOC * S], FP16)  # [d%128, kd, (b,s)]
                QT_sb = actsp.tile([128, KD, B_LOC * S], FP16)  # [j%128, kj, (b,s)]
                KT_sb = actsp.tile([128, KD, B_LOC * S], FP16)
                V_sb = actsp.tile([128, B_LOC, H, DHP], BF16)  # [s, b, h, dh+1]
                nc.vector.memset(V_sb[:, :, :, DH:DHP], 1.0)
                th_sb = actsp.tile([128, B_LOC, D], FP16)      # tanh(attn) [s, b, j]
                thsel_sb = actsp.tile([128, KD, I_LOC], FP16)  # [d%128, kd, i]

                # --- stage 1 of the batch pipeline: transpose+pe, QKV, S^T ---
                def stage1(b):
                    bs = slice(b * S, (b + 1) * S)
                    for kd in range(KD):
                        tp_ps = ps_t.tile([128, 128], FP16, tag="t")
                        nc.tensor.transpose(tp_ps[:, :],
                                            gath_tiles[b][:, kd * 128:(kd + 1) * 128],
                                            ident_sb[:, :])
                        nc.vector.tensor_add(
                            out=currT_sb[:, kd, b * S:(b + 1) * S],
                            in0=tp_ps[:, :], in1=peT_sb[:, kd, :])
                    # The bias TSP for kj runs while kj+1's matmuls stream, so
                    # the DVE adds stay off the PE critical path.
                    q_ps = ps_qkv.tile([128, KD * S], FP32, tag="qkv")
                    k_ps = ps_qkv.tile([128, KD * S], FP32, tag="qkv")
                    for kj in range(KD):
                        for kd in range(KD):
                            nc.tensor.matmul(q_ps[:, kj * S:(kj + 1) * S],
                                             wqt_sb[:, kd, kj * 128:(kj + 1) * 128],
                                             currT_sb[:, kd, bs],
                                             start=(kd == 0), stop=(kd == KD - 1))
                        nc.vector.tensor_scalar_add(QT_sb[:, kj, bs],
                                                    q_ps[:, kj * S:(kj + 1) * S],
                                                    b3_sb[:, kj, 0:1])
                        for kd in range(KD):
                            nc.tensor.matmul(k_ps[:, kj * S:(kj + 1) * S],
                                             wkt_sb[:, kd, kj * 128:(kj + 1) * 128],
                                             currT_sb[:, kd, bs],
                                             start=(kd == 0), stop=(kd == KD - 1))
                        nc.vector.tensor_scalar_add(KT_sb[:, kj, bs],
                                                    k_ps[:, kj * S:(kj + 1) * S],
                                                    b3_sb[:, kj, 1:2])
                    v_ps = ps_qkv.tile([128, D], FP32, tag="qkv")
                    for kd in range(KD):
                        nc.tensor.matmul(v_ps[:, :],
                                         currT_sb[:, kd, bs],
                                         wvt_sb[:, kd, :],
                                         start=(kd == 0), stop=(kd == KD - 1))
                    nc.vector.tensor_add(
                        out=V_sb[:, b, :, 0:DH],
                        in0=v_ps[:, :].rearrange("p (h d) -> p h d", d=DH),
                        in1=bvb_sb[:, :].rearrange("p (h d) -> p h d", d=DH))
                    # S^T scores: heads with row-half 0 in one PSUM bank, 1 in
                    # the other (same-bank matmuls must share a row group)
                    s_ps0 = ps_sav.tile([128, 4 * S], FP32, tag="sav")
                    s_ps1 = ps_sav.tile([128, 4 * S], FP32, tag="sav")
                    for h in range(H):
                        kj, half = divmod(h, 2)
                        rows = slice(half * 64, (half + 1) * 64)
                        s_ps_h = s_ps0 if half == 0 else s_ps1
                        o = (h // 2) * S
                        nc.tensor.matmul(s_ps_h[:, o:o + S],
                                         KT_sb[rows, kj, bs], QT_sb[rows, kj, bs],
                                         start=True, stop=True)
                    p_sb = php.tile([128, H * S], BF16, tag="p", bufs=3)  # P^T slots
                    nc.scalar.activation(p_sb[:, 0:4 * S], s_ps0[:, :], ACTF.Exp,
                                         bias=shatt_sb[:, :1])
                    nc.scalar.activation(p_sb[:, 4 * S:8 * S], s_ps1[:, :], ACTF.Exp,
                                         bias=shatt_sb[:, :1])
                    return p_sb

                # --- stage 2: AV', normalize, tanh, select ---
                def stage2(b, p_sb):
                    av_ps0 = ps_sav.tile([128, 4 * DHP], FP32, tag="sav")
                    av_ps1 = ps_sav.tile([128, 4 * DHP], FP32, tag="sav")
                    last_av = [None, None]
                    for h in range(H):
                        sl = (h % 2) * 4 + h // 2  # slot index of head h in p_sb
                        av_ps = av_ps0 if h < 4 else av_ps1
                        last_av[h // 4] = nc.tensor.matmul(
                            av_ps[:, (h % 4) * DHP:(h % 4 + 1) * DHP],
                            p_sb[:, sl * S:(sl + 1) * S],
                            V_sb[:, b, h, :],
                            start=True, stop=True)
                    rec_sb = smallp.tile([128, H], FP32, tag="rec")
                    for g in range(2):
                        av_ps = av_ps0 if g == 0 else av_ps1
                        r_i = nc.vector.reciprocal(
                            rec_sb[:, g * 4:(g + 1) * 4],
                            av_ps[:, :].rearrange("p (h d) -> p h d", d=DHP)[:, :, DH])
                        add_dep_helper(r_i.ins, last_av[g].ins, reason="rec after AV bank")
                    avn_sb = php.tile([128, H, DH], FP32, tag="avn", bufs=3)
                    for g in range(2):
                        av_ps = av_ps0 if g == 0 else av_ps1
                        m_i = nc.vector.tensor_mul(
                            out=avn_sb[:, g * 4:(g + 1) * 4, :],
                            in0=av_ps[:, :].rearrange("p (h d) -> p h d", d=DHP)[:, :, 0:DH],
                            in1=rec_sb[:, g * 4:(g + 1) * 4]
                                .rearrange("p (h one) -> p h one", one=1)
                                .to_broadcast([128, 4, DH]))
                        add_dep_helper(m_i.ins, last_av[g].ins,
                                       reason="attn bank read after all AV writes")
                    nc.scalar.activation(th_sb[:, b, :],
                                         avn_sb[:, :, :].rearrange("p h d -> p (h d)"),
                                         ACTF.Tanh)

                # --- stage 3: select this batch's mask positions ---
                def stage3(b):
                    ts_ps = ps_t.tile([128, KD, NM], FP32, tag="t")
                    for kd in range(KD):
                        nc.tensor.matmul(ts_ps[:, kd, :],
                                         th_sb[:, b, kd * 128:(kd + 1) * 128],
                                         sel_sb[:, b, :],
                                         start=True, stop=True)
                    nc.vector.tensor_copy(
                        out=thsel_sb[:, :, :]
                            .rearrange("p kd (b m) -> p kd b m", b=B_LOC)[:, :, b, :],
                        in_=ts_ps[:, :, :])

                # two-deep software pipeline: PE never waits on the scalar
                # engine's exp (stage1->stage2 edge) or tanh (stage2->stage3)
                pipe = []
                for b in range(B_LOC):
                    p_sb = stage1(b)
                    if pipe:
                        stage2(*pipe[-1])
                    if len(pipe) >= 2:
                        stage3(pipe[-2][0])
                    pipe.append((b, p_sb))
                stage2(*pipe[-1])
                stage3(pipe[-2][0])
                stage3(pipe[-1][0])

                # t2 projection -> G^T [d, i_loc], all four mj blocks in one
                # SBUF tile so the AllGather input is a single DMA
                gt_sb = smallp.tile([128, KD, I_LOC], FP16, tag="gt", bufs=1)
                for mj in range(KD):
                    g_ps = ps_t.tile([128, I_LOC], FP32, tag="t")
                    for kd in range(KD):
                        nc.tensor.matmul(g_ps[:, :],
                                         t2wt_sb[:, kd, mj * 128:(mj + 1) * 128],
                                         thsel_sb[:, kd, :],
                                         start=(kd == 0), stop=(kd == KD - 1))
                    nc.vector.tensor_scalar_add(gt_sb[:, mj, :], g_ps[:, :],
                                                b3_sb[:, mj, 2:3])
                nc.sync.dma_start(
                    out=ag_g_in[:].rearrange("(mj p i) -> p mj i", p=128, mj=KD),
                    in_=gt_sb[:, :, :])

                # ---- AllGather G (the only collective) ----
                if sim_local:
                    nc.sync.dma_start(
                        out=ag_g_out[:].rearrange("(c x) -> c x", c=N_CORES),
                        in_=ag_g_in[:].rearrange("(one x) -> one x", one=1)
                            .to_broadcast([N_CORES, D * I_LOC]))
                else:
                    nc.gpsimd.collective_compute(
                        "AllGather", mybir.AluOpType.bypass,
                        replica_groups=[list(range(N_CORES))],
                        ins=[ag_g_in[:].opt()], outs=[ag_g_out[:].opt()],
                    )
                ag_g_view = ag_g_out[:].rearrange("(c kd p i) -> p kd c i", p=128, kd=KD, i=I_LOC)
                for kd in range(KD):
                    nc.sync.dma_start(out=GT_sb[:, kd, :, :], in_=ag_g_view[:, kd, :, :])

            # ================= Phase B: scores -> exp -> DMA out =================
            with (
                tc.tile_pool(name="ps_sc", bufs=8, space="PSUM") as ps_sc,
                tc.tile_pool(name="esb", bufs=4) as esbp,
            ):
                gt_view = GT_sb[:, :, :, :].rearrange("p kd c i -> p kd (c i)")
                for v in range(NCH):
                    for mi in range(KD):
                        sc_ps = ps_sc.tile([128, VCH], FP32, tag="sc")
                        for kd in range(KD):
                            nc.tensor.matmul(sc_ps[:, :],
                                             gt_view[:, kd, mi * 128:(mi + 1) * 128],
                                             cand_sb[:, kd, v * VCH:(v + 1) * VCH],
                                             start=(kd == 0), stop=(kd == KD - 1))
                        e_sb = esbp.tile([128, VCH], BF16, tag="e")
                        nc.scalar.activation(e_sb[:, :], sc_ps[:, :],
                                             ACTF.Exp, bias=shsc_sb[:, :1])
                        nc.sync.dma_start(
                            out=out_e.ap()[mi * 128:(mi + 1) * 128, v * VCH:(v + 1) * VCH],
                            in_=e_sb[:, :])
    nc.compile()
    return nc


_NC_CACHE = None


def _get_nc():
    global _NC_CACHE
    if _NC_CACHE is None:
        _NC_CACHE = build()
    return _NC_CACHE


def prepare_in_maps(inputs):
    emb = np.asarray(inputs["emb"], dtype=np.float32)
    emb16 = np.ascontiguousarray(emb.astype(np.float16))
    mask_curr = np.asarray(inputs["mask_curr_traj_grid"]).astype(np.int32)
    mask_pos = np.asarray(inputs["mask_pos"]).astype(np.int32)
    wqt = np.ascontiguousarray(np.asarray(inputs["c_wq"], dtype=np.float32).T.astype(np.float16))
    wkt = np.ascontiguousarray(np.asarray(inputs["c_wk"], dtype=np.float32).T.astype(np.float16))
    wvt = np.ascontiguousarray(np.asarray(inputs["c_wv"], dtype=np.float32).T.astype(np.float16))
    t2wt = np.ascontiguousarray(np.asarray(inputs["t2_w"], dtype=np.float32).T.astype(np.float16))
    bias3 = np.ascontiguousarray(np.stack(
        [np.asarray(inputs["c_bq"], dtype=np.float32),
         np.asarray(inputs["c_bk"], dtype=np.float32),
         np.asarray(inputs["t2_b"], dtype=np.float32)], axis=1))
    bv = np.asarray(inputs["c_bv"], dtype=np.float32)
    peT = np.ascontiguousarray(_positional_embedding(D, S).T)
    ident = np.eye(128, dtype=np.float16)

    cand = emb[2:]
    in_maps = []
    for c in range(N_CORES):
        lo = c * VSH
        hi = min((c + 1) * VSH, VOCAB)
        shard = np.zeros((VSH, D), dtype=np.float16)
        shard[: hi - lo] = cand[lo:hi].astype(np.float16)
        candT_c = np.ascontiguousarray(shard.T)
        mp = mask_pos[c * B_LOC:(c + 1) * B_LOC]  # [B_LOC, NM]
        sel_c = np.zeros((B_LOC, S, NM), dtype=np.float16)
        for b in range(B_LOC):
            sel_c[b, mp[b], np.arange(NM)] = 1.0
        in_maps.append(dict(
            emb16=emb16,
            candT=candT_c,
            idx=np.ascontiguousarray(mask_curr[c * B_LOC:(c + 1) * B_LOC].reshape(-1)),
            peT=peT, wqt=wqt, wkt=wkt, wvt=wvt, t2wt=t2wt,
            bias3=bias3, bv=bv,
            sel=sel_c, ident=ident,
        ))
    return in_maps


def assemble_output(results):
    # host-side log_softmax epilogue: out = ln(e) - ln(sum e); the exp sums are
    # computed here directly from the e chunks (valid columns only, so the
    # padded vocab columns never enter the denominator)
    es = []
    s_tot = np.zeros(I_TOT, dtype=np.float64)
    for c in range(N_CORES):
        lo = c * VSH
        hi = min((c + 1) * VSH, VOCAB)
        e = np.asarray(results[c]["out_e"][:, : hi - lo], dtype=np.float32)
        s_tot += e.sum(axis=1, dtype=np.float64)
        es.append(e)
    ln_s = np.log(s_tot).astype(np.float32)[:, None]
    parts = [np.log(np.maximum(e, 1e-38)) - ln_s for e in es]
    return np.ascontiguousarray(np.concatenate(parts, axis=1))


def kernel(**inputs):
    nc = _get_nc()
    in_maps = prepare_in_maps(inputs)
    res = run_bass_kernel_spmd(nc, in_maps, core_ids=list(range(N_CORES)))
    return assemble_output(res.results)
